# revision 34
# baseline (speedup 1.0000x reference)
"""Trainium2 Bass kernel for nn_DifferentiableHMM_Centered.

Computes, for x (S,B) f32 and gumbel_noise (S,B,3) f32:
  norm_copy[s,b] = all_means[argmax_k y_k]   (straight-through gumbel argmax)
  loss = 0.1 * mean((states[row]-states[col])**2)
       = 0.1 * 2 * #{(e,b): argmax differs} / (E*B*3)
where y_k = (log N(x; mu_k, sigma_k) + g_k)/tau.  The Viterbi trellis in the
reference is dead code (outputs never consume it) and is skipped.

argmax_k y_k == argmax_k (g_k - ((x-mu_k)*r'_k)^2 + d_k), r'_k = 1/(sigma_k*sqrt(2)),
d_k = -log sigma_k (d_k constant when sigmas equal -> dropped).

Sharding: bins split across 8 cores (phase A elementwise is trivially parallel;
phase B edge-mismatch counting is local per bin-slice, edges replicated).
Per core: codes (argmax in {0,1,2}) are packed 2 bits/bin into int32 words
(12+4 bins per word via an fp32 mult-add scan), written to a DRAM table with
256B rows, row-gathered per edge endpoint with dma_gather, XOR'd and
SWAR-popcounted to mismatch counts.
"""
import numpy as np

import concourse.bacc as bacc
import concourse.mybir as mybir
from concourse.tile import TileContext
from concourse import bass_utils
from concourse.library_config import mlp

AL = mybir.AluOpType
AF = mybir.ActivationFunctionType
DT = mybir.dt

S = 4096            # spots
B = 4096            # bins (total)
E = 24576           # edges
NCORES = 8
BS = B // NCORES    # bins per core = 512
P = 128
NG = S // P         # spot groups = 32
CHUNK_T = 4         # spot groups per phase-A chunk
NCHUNK = NG // CHUNK_T
FD = CHUNK_T * BS   # phase-A chunk free dim = 2048
WPR = 64            # int32 words per code row (256B, only first BS//16=32 used)
WUSED = BS // 16    # 32 packed words per row
ECH = E // 4        # edges per phase-B chunk = 6144
EPP = ECH // P      # edge rows per partition per chunk = 48

_CACHE = {}


def _imm(v):
    return mybir.ImmediateValue(dtype=DT.int32, value=int(v))


def _ts(eng, out, in0, s1, op0, s2, op1, accum_out=None):
    """tensor_scalar with int32 immediates; op0/op1 same class (both bitvec)."""
    outs = [eng.lower_ap(out)]
    if accum_out is not None:
        outs.append(eng.lower_ap(accum_out))
    return eng.add_instruction(mybir.InstTensorScalarPtr(
        name=eng.bass.get_next_instruction_name(),
        op0=op0, op1=op1,
        ins=[eng.lower_ap(in0), _imm(s1), _imm(s2)],
        outs=outs))


def _stt(eng, out, in0, s, in1, op0, op1):
    return eng.add_instruction(mybir.InstTensorScalarPtr(
        name=eng.bass.get_next_instruction_name(),
        is_scalar_tensor_tensor=True, op0=op0, op1=op1,
        ins=[eng.lower_ap(in0), _imm(s), eng.lower_ap(in1)],
        outs=[eng.lower_ap(out)]))


PHASE_B = True


def _build(means, rp, dk, stds_equal):
    """Build the SPMD program (same for all cores; bins slice differs only in
    the input data)."""
    nc = bacc.Bacc(None, target_bir_lowering=False, debug=True)

    x_s = nc.dram_tensor("x_s", [S, BS], DT.float32, kind="ExternalInput")
    g_s = nc.dram_tensor("g_s", [3, S, BS], DT.float32, kind="ExternalInput")
    idx_r = nc.dram_tensor("idx_r", [P, E // 16], DT.int16, kind="ExternalInput")
    idx_c = nc.dram_tensor("idx_c", [P, E // 16], DT.int16, kind="ExternalInput")
    pat = nc.dram_tensor("pat", [P, FD], DT.float32, kind="ExternalInput")
    norm = nc.dram_tensor("norm", [S, BS], DT.float32, kind="ExternalOutput")
    partials = nc.dram_tensor("partials", [P, 8], DT.float32, kind="ExternalOutput")
    codes = nc.dram_tensor("codes", [S, WPR], DT.int32)

    x_r = x_s.rearrange("(t p) b -> t p b", p=P)
    g_r = g_s.rearrange("k (t p) b -> k t p b", p=P)
    n_r = norm.rearrange("(t p) b -> t p b", p=P)
    c_r = codes.rearrange("(t p) w -> t p w", p=P)

    with TileContext(nc) as tc:
        nc.gpsimd.load_library(mlp)
        with tc.tile_pool(name="outer", bufs=1) as outer:
            # constants
            bias_t = [outer.tile([P, 1], DT.float32, tag=f"bias{k}", name=f"bias{k}") for k in range(3)]
            for k in range(3):
                nc.vector.memset(bias_t[k][:], float(np.float32(-np.float32(means[k]) * np.float32(rp[k]))))
            pat_t = outer.tile([P, FD], DT.float32, tag="pat")
            nc.sync.dma_start(pat_t[:], pat[:])
            idxr_t = outer.tile([P, E // 16], DT.int16, tag="idxr")
            idxc_t = outer.tile([P, E // 16], DT.int16, tag="idxc")
            nc.sync.dma_start(idxr_t[:], idx_r[:])
            nc.sync.dma_start(idxc_t[:], idx_c[:])
            partial_t = outer.tile([P, 8], DT.float32, tag="partial")
            nc.vector.memset(partial_t[:], 0.0)

            # ---------------- phase A ----------------
            with tc.tile_pool(name="pa", bufs=2) as pa:
                for ci in range(NCHUNK):
                    t0 = ci * CHUNK_T
                    xt = pa.tile([P, CHUNK_T, BS], DT.float32, tag="x", bufs=3)
                    nc.sync.dma_start(xt[:], x_r[t0:t0 + CHUNK_T].rearrange("t p b -> p t b"))

                    # B_k = sq_k + (-g_k): ACT Square, then the gumbel-plane
                    # load DMA accumulates (CCE add) onto it -- the subtraction
                    # costs zero DVE passes.  Host supplies negated planes
                    # (with -d_k folded in when sigmas differ).  argmax b ==
                    # argmin B, so comparisons below invert (min / is_gt).
                    sq = []
                    for k in range(3):
                        s_k = pa.tile([P, CHUNK_T, BS], DT.float32, tag=f"sq{k}", name=f"sq{k}", bufs=3)
                        nc.scalar.activation(s_k[:], xt[:], AF.Square,
                                             bias=bias_t[k][:], scale=float(rp[k]))
                        nc.gpsimd.dma_start(s_k[:], g_r[k, t0:t0 + CHUNK_T].rearrange("t p b -> p t b"),
                                            accum_op=AL.add)
                        sq.append(s_k)

                    # code in {0,1,2}; first-index-wins tie-break:
                    # code = [B0 > min(B1,B2)] * (1 + [B1 > B2])
                    m12 = pa.tile([P, CHUNK_T, BS], DT.float32, tag="m12", bufs=1)
                    nc.vector.tensor_tensor(out=m12[:], in0=sq[1][:], in1=sq[2][:], op=AL.min)
                    c12i = pa.tile([P, CHUNK_T, BS], DT.float32, tag="c12i", bufs=1)
                    nc.vector.tensor_tensor(out=c12i[:], in0=sq[1][:], in1=sq[2][:], op=AL.is_gt)
                    c0i = pa.tile([P, CHUNK_T, BS], DT.float32, tag="c0i", bufs=1)
                    nc.vector.tensor_tensor(out=c0i[:], in0=sq[0][:], in1=m12[:], op=AL.is_gt)
                    code = pa.tile([P, CHUNK_T, BS], DT.float32, tag="code")
                    nc.vector.scalar_tensor_tensor(out=code[:], in0=c12i[:], scalar=1.0,
                                                   in1=c0i[:], op0=AL.add, op1=AL.mult)

                    # norm_copy = means[code] = scale*code + bias (means affine in code
                    # only when means are affine; general: two predicated copies)
                    ncv = pa.tile([P, CHUNK_T, BS], DT.float32, tag="ncv")
                    nc.scalar.activation(ncv[:], code[:], AF.Copy,
                                         bias=float(_affine_bias(means)), scale=float(_affine_scale(means)))
                    nc.sync.dma_start(n_r[t0:t0 + CHUNK_T].rearrange("t p b -> p t b"), ncv[:])

                    # pack codes 2b/bin: scan resets at j%16 in {0,12}
                    sc = pa.tile([P, FD], DT.float32, tag="scan", bufs=1)
                    nc.vector.tensor_tensor_scan(
                        out=sc[:], data0=pat_t[:], data1=code[:].rearrange("p t b -> p (t b)"),
                        initial=0.0, op0=AL.mult, op1=AL.add)
                    lo = pa.tile([P, FD // 16], DT.int32, tag="lo")
                    hi = pa.tile([P, FD // 16], DT.int32, tag="hi")
                    nc.scalar.activation(lo[:], sc[:, 11::16], AF.Copy, bias=0.0, scale=1.0)
                    nc.scalar.activation(hi[:], sc[:, 15::16], AF.Copy, bias=0.0, scale=1.0)
                    wds = pa.tile([P, FD // 16], DT.int32, tag="wds")
                    _stt(nc.vector, wds[:], hi[:], 24, lo[:], AL.logical_shift_left, AL.bitwise_or)
                    nc.sync.dma_start(
                        c_r[t0:t0 + CHUNK_T, :, 0:WUSED].rearrange("t p w -> p t w"),
                        wds[:].rearrange("p (t w) -> p t w", t=CHUNK_T))

            tc.strict_bb_all_engine_barrier()

            # ---------------- phase B ----------------
            with tc.tile_pool(name="pb", bufs=2) as pb:
                for ch in (range(4) if PHASE_B else []):
                    rg = pb.tile([P, EPP, WPR], DT.int32, tag="rg")
                    cg = pb.tile([P, EPP, WPR], DT.int32, tag="cg")
                    i0 = ch * (ECH // 16)
                    i1 = (ch + 1) * (ECH // 16)
                    nc.gpsimd.dma_gather(rg[:], codes[:], idxr_t[:, i0:i1], ECH, ECH, WPR, single_packet=False)
                    nc.gpsimd.dma_gather(cg[:], codes[:], idxc_t[:, i0:i1], ECH, ECH, WPR, single_packet=False)

                    nw = EPP * WUSED  # 1536 real words per partition
                    t = pb.tile([P, nw], DT.int32, tag="t")
                    tv = t[:].rearrange("p (a b) -> p a b", a=EPP)
                    nc.vector.tensor_tensor(out=tv, in0=rg[:, :, 0:WUSED], in1=cg[:, :, 0:WUSED],
                                            op=AL.bitwise_xor)
                    # a = t | t>>1: bit 2j = [field j mismatched] (odd bits garbage,
                    # excluded by the 0x11 masks below)
                    _stt(nc.vector, t[:], t[:], 1, t[:], AL.logical_shift_right, AL.bitwise_or)
                    # nibble counts <=2
                    v2 = pb.tile([P, nw], DT.int32, tag="v2")
                    v1 = pb.tile([P, nw], DT.int32, tag="v1")
                    _ts(nc.vector, v2[:], t[:], 2, AL.logical_shift_right, 0x11111111, AL.bitwise_and)
                    _ts(nc.vector, v1[:], t[:], 0x11111111, AL.bitwise_and, 0, AL.bitwise_or)
                    # int32 DVE adds go through fp32; SWAR sums never carry across
                    # 16-bit halves, so add on int16 views (values < 2^24, exact)
                    nc.vector.tensor_tensor(out=t[:].bitcast(DT.int16), in0=v1[:].bitcast(DT.int16),
                                            in1=v2[:].bitcast(DT.int16), op=AL.add)
                    # byte counts <=4
                    _ts(nc.vector, v2[:], t[:], 4, AL.logical_shift_right, 0x0F0F0F0F, AL.bitwise_and)
                    _ts(nc.vector, v1[:], t[:], 0x0F0F0F0F, AL.bitwise_and, 0, AL.bitwise_or)
                    nc.vector.tensor_tensor(out=t[:].bitcast(DT.int16), in0=v1[:].bitcast(DT.int16),
                                            in1=v2[:].bitcast(DT.int16), op=AL.add)
                    # sum all bytes on ACT (accum_out); counts <= 24576, f32-exact
                    dump = pb.tile([P, EPP * WUSED * 4], DT.int8, tag="dump")
                    nc.scalar.activation(dump[:], t[:].bitcast(DT.int8), AF.Copy,
                                         bias=0.0, scale=1.0, accum_out=partial_t[:, ch:ch + 1])
            nc.sync.dma_start(partials[:], partial_t[:])
    nc.compile()
    return nc


def _affine_scale(means):
    # means[k] = scale*k + bias requires affine means; true for (-mu, 0, mu)?
    # General 3-point: only if means[2]-means[1] == means[1]-means[0].
    return np.float32(means[1] - means[0])


def _affine_bias(means):
    return np.float32(means[0])


def _means_affine(means):
    return np.float32(means[2]) - np.float32(means[1]) == np.float32(means[1]) - np.float32(means[0])


def kernel(x, edge_index, gumbel_noise, state_means, log_stds, transition_logits, start_logits):
    x = np.ascontiguousarray(np.asarray(x, dtype=np.float32))
    g = np.ascontiguousarray(np.asarray(gumbel_noise, dtype=np.float32))
    ei = np.asarray(edge_index)
    sm = np.asarray(state_means, dtype=np.float32)
    ls = np.asarray(log_stds, dtype=np.float32)

    means = np.array([sm[0], np.float32(0.0), sm[1]], dtype=np.float32)
    stds = (np.exp(ls) + np.float32(1e-6)).astype(np.float32)
    stds_equal = bool(np.all(stds == stds[0]))
    r = (np.float32(1.0) / stds).astype(np.float32)
    rp = (r.astype(np.float64) / np.sqrt(np.float64(2.0))).astype(np.float32)
    dk = (-np.log(stds)).astype(np.float32)  # only used when stds differ
    assert _means_affine(means), "non-affine means need a predicated-copy path"

    key = (means.tobytes(), rp.tobytes(), dk.tobytes(), stds_equal)
    if key not in _CACHE:
        _CACHE[key] = _build(means, rp, dk, stds_equal)
    nc = _CACHE[key]

    # host-side input prep
    pat_row = np.where(np.isin(np.arange(FD) % 16, (0, 12)), np.float32(0.0), np.float32(4.0))
    pat_full = np.tile(pat_row[None, :], (P, 1)).astype(np.float32)

    def g_planes(gg, b0, b1):
        pl = np.ascontiguousarray(-gg[:, b0:b1, :].transpose(2, 0, 1))
        if not stds_equal:
            pl = pl - dk.reshape(3, 1, 1).astype(np.float32)
        return np.ascontiguousarray(pl.astype(np.float32))

    def wrap_idx(a):
        # per phase-B chunk of ECH edges: idx j of chunk at [j%16, ch*(ECH//16)+j//16]
        cols = []
        for ch in range(4):
            blk = a[ch * ECH:(ch + 1) * ECH].reshape(ECH // 16, 16).T
            cols.append(blk)
        return np.tile(np.concatenate(cols, axis=1), (P // 16, 1)).astype(np.int16)

    idx_r = wrap_idx(ei[0].astype(np.int64))
    idx_c = wrap_idx(ei[1].astype(np.int64))

    in_maps = []
    for c in range(NCORES):
        b0, b1 = c * BS, (c + 1) * BS
        in_maps.append({
            "x_s": np.ascontiguousarray(x[:, b0:b1]),
            "g_s": g_planes(g, b0, b1),
            "idx_r": idx_r,
            "idx_c": idx_c,
            "pat": pat_full,
        })

    global LAST_RESULT, LAST_IN_MAPS
    LAST_IN_MAPS = in_maps
    res = bass_utils.run_bass_kernel_spmd(nc, in_maps, core_ids=list(range(NCORES)))
    LAST_RESULT = res
    outs = res.results

    norm_copy = np.concatenate([outs[c]["norm"] for c in range(NCORES)], axis=1)
    total = sum(int(outs[c]["partials"][:, 0:4].astype(np.int64).sum()) for c in range(NCORES))
    loss = np.float32(np.float64(0.1) * 2.0 * total / (E * B * 3))
    return norm_copy, loss


# revision 35
# speedup vs baseline: 1.0190x; 1.0190x over previous
"""Trainium2 Bass kernel for nn_DifferentiableHMM_Centered.

Computes, for x (S,B) f32 and gumbel_noise (S,B,3) f32:
  norm_copy[s,b] = all_means[argmax_k y_k]   (straight-through gumbel argmax)
  loss = 0.1 * mean((states[row]-states[col])**2)
       = 0.1 * 2 * #{(e,b): argmax differs} / (E*B*3)
where y_k = (log N(x; mu_k, sigma_k) + g_k)/tau.  The Viterbi trellis in the
reference is dead code (outputs never consume it) and is skipped.

argmax_k y_k == argmax_k (g_k - ((x-mu_k)*r'_k)^2 + d_k), r'_k = 1/(sigma_k*sqrt(2)),
d_k = -log sigma_k (d_k constant when sigmas equal -> dropped).

Sharding: bins split across 8 cores (phase A elementwise is trivially parallel;
phase B edge-mismatch counting is local per bin-slice, edges replicated).
Per core: codes (argmax in {0,1,2}) are packed 2 bits/bin into int32 words
(12+4 bins per word via an fp32 mult-add scan), written to a DRAM table with
256B rows, row-gathered per edge endpoint with dma_gather, XOR'd and
SWAR-popcounted to mismatch counts.
"""
import numpy as np

import concourse.bacc as bacc
import concourse.mybir as mybir
from concourse.tile import TileContext
from concourse import bass_utils
from concourse.library_config import mlp

AL = mybir.AluOpType
AF = mybir.ActivationFunctionType
DT = mybir.dt

S = 4096            # spots
B = 4096            # bins (total)
E = 24576           # edges
NCORES = 8
BS = B // NCORES    # bins per core = 512
P = 128
NG = S // P         # spot groups = 32
CHUNK_T = 4         # spot groups per phase-A chunk
NCHUNK = NG // CHUNK_T
FD = CHUNK_T * BS   # phase-A chunk free dim = 2048
WPR = 64            # int32 words per code row (256B, only first BS//16=32 used)
WUSED = BS // 16    # 32 packed words per row
ECH = E // 8        # edges per phase-B chunk = 3072
EPP = ECH // P      # edge rows per partition per chunk = 48

_CACHE = {}


def _imm(v):
    return mybir.ImmediateValue(dtype=DT.int32, value=int(v))


def _ts(eng, out, in0, s1, op0, s2, op1, accum_out=None):
    """tensor_scalar with int32 immediates; op0/op1 same class (both bitvec)."""
    outs = [eng.lower_ap(out)]
    if accum_out is not None:
        outs.append(eng.lower_ap(accum_out))
    return eng.add_instruction(mybir.InstTensorScalarPtr(
        name=eng.bass.get_next_instruction_name(),
        op0=op0, op1=op1,
        ins=[eng.lower_ap(in0), _imm(s1), _imm(s2)],
        outs=outs))


def _stt(eng, out, in0, s, in1, op0, op1):
    return eng.add_instruction(mybir.InstTensorScalarPtr(
        name=eng.bass.get_next_instruction_name(),
        is_scalar_tensor_tensor=True, op0=op0, op1=op1,
        ins=[eng.lower_ap(in0), _imm(s), eng.lower_ap(in1)],
        outs=[eng.lower_ap(out)]))


PHASE_B = True


def _build(means, rp, dk, stds_equal):
    """Build the SPMD program (same for all cores; bins slice differs only in
    the input data)."""
    nc = bacc.Bacc(None, target_bir_lowering=False, debug=True)

    x_s = nc.dram_tensor("x_s", [S, BS], DT.float32, kind="ExternalInput")
    g_s = nc.dram_tensor("g_s", [3, S, BS], DT.float32, kind="ExternalInput")
    idx_r = nc.dram_tensor("idx_r", [P, E // 16], DT.int16, kind="ExternalInput")
    idx_c = nc.dram_tensor("idx_c", [P, E // 16], DT.int16, kind="ExternalInput")
    pat = nc.dram_tensor("pat", [P, FD], DT.float32, kind="ExternalInput")
    norm = nc.dram_tensor("norm", [S, BS], DT.float32, kind="ExternalOutput")
    partials = nc.dram_tensor("partials", [P, 8], DT.float32, kind="ExternalOutput")
    codes = nc.dram_tensor("codes", [S, WPR], DT.int32)

    x_r = x_s.rearrange("(t p) b -> t p b", p=P)
    g_r = g_s.rearrange("k (t p) b -> k t p b", p=P)
    n_r = norm.rearrange("(t p) b -> t p b", p=P)
    c_r = codes.rearrange("(t p) w -> t p w", p=P)

    with TileContext(nc) as tc:
        nc.gpsimd.load_library(mlp)
        with tc.tile_pool(name="outer", bufs=1) as outer:
            # constants
            bias_t = [outer.tile([P, 1], DT.float32, tag=f"bias{k}", name=f"bias{k}") for k in range(3)]
            for k in range(3):
                nc.vector.memset(bias_t[k][:], float(np.float32(-np.float32(means[k]) * np.float32(rp[k]))))
            pat_t = outer.tile([P, FD], DT.float32, tag="pat")
            nc.sync.dma_start(pat_t[:], pat[:])
            idxr_t = outer.tile([P, E // 16], DT.int16, tag="idxr")
            idxc_t = outer.tile([P, E // 16], DT.int16, tag="idxc")
            nc.sync.dma_start(idxr_t[:], idx_r[:])
            nc.sync.dma_start(idxc_t[:], idx_c[:])
            partial_t = outer.tile([P, 8], DT.float32, tag="partial")
            nc.vector.memset(partial_t[:], 0.0)

            # ---------------- phase A ----------------
            with tc.tile_pool(name="pa", bufs=2) as pa:
                for ci in range(NCHUNK):
                    t0 = ci * CHUNK_T
                    xt = pa.tile([P, CHUNK_T, BS], DT.float32, tag="x", bufs=3)
                    nc.sync.dma_start(xt[:], x_r[t0:t0 + CHUNK_T].rearrange("t p b -> p t b"))

                    # B_k = sq_k + (-g_k): ACT Square, then the gumbel-plane
                    # load DMA accumulates (CCE add) onto it -- the subtraction
                    # costs zero DVE passes.  Host supplies negated planes
                    # (with -d_k folded in when sigmas differ).  argmax b ==
                    # argmin B, so comparisons below invert (min / is_gt).
                    sq = []
                    for k in range(3):
                        s_k = pa.tile([P, CHUNK_T, BS], DT.float32, tag=f"sq{k}", name=f"sq{k}", bufs=3)
                        nc.scalar.activation(s_k[:], xt[:], AF.Square,
                                             bias=bias_t[k][:], scale=float(rp[k]))
                        nc.gpsimd.dma_start(s_k[:], g_r[k, t0:t0 + CHUNK_T].rearrange("t p b -> p t b"),
                                            accum_op=AL.add)
                        sq.append(s_k)

                    # code in {0,1,2}; first-index-wins tie-break:
                    # code = [B0 > min(B1,B2)] * (1 + [B1 > B2])
                    m12 = pa.tile([P, CHUNK_T, BS], DT.float32, tag="m12", bufs=1)
                    nc.vector.tensor_tensor(out=m12[:], in0=sq[1][:], in1=sq[2][:], op=AL.min)
                    c12i = pa.tile([P, CHUNK_T, BS], DT.float32, tag="c12i", bufs=1)
                    nc.vector.tensor_tensor(out=c12i[:], in0=sq[1][:], in1=sq[2][:], op=AL.is_gt)
                    c0i = pa.tile([P, CHUNK_T, BS], DT.float32, tag="c0i", bufs=1)
                    nc.vector.tensor_tensor(out=c0i[:], in0=sq[0][:], in1=m12[:], op=AL.is_gt)
                    code = pa.tile([P, CHUNK_T, BS], DT.float32, tag="code")
                    nc.vector.scalar_tensor_tensor(out=code[:], in0=c12i[:], scalar=1.0,
                                                   in1=c0i[:], op0=AL.add, op1=AL.mult)

                    # norm_copy = means[code] = scale*code + bias (means affine in code
                    # only when means are affine; general: two predicated copies)
                    ncv = pa.tile([P, CHUNK_T, BS], DT.float32, tag="ncv")
                    nc.scalar.activation(ncv[:], code[:], AF.Copy,
                                         bias=float(_affine_bias(means)), scale=float(_affine_scale(means)))
                    nc.sync.dma_start(n_r[t0:t0 + CHUNK_T].rearrange("t p b -> p t b"), ncv[:])

                    # pack codes 2b/bin: scan resets at j%16 in {0,12}
                    sc = pa.tile([P, FD], DT.float32, tag="scan", bufs=1)
                    nc.vector.tensor_tensor_scan(
                        out=sc[:], data0=pat_t[:], data1=code[:].rearrange("p t b -> p (t b)"),
                        initial=0.0, op0=AL.mult, op1=AL.add)
                    lo = pa.tile([P, FD // 16], DT.int32, tag="lo")
                    hi = pa.tile([P, FD // 16], DT.int32, tag="hi")
                    nc.scalar.activation(lo[:], sc[:, 11::16], AF.Copy, bias=0.0, scale=1.0)
                    nc.scalar.activation(hi[:], sc[:, 15::16], AF.Copy, bias=0.0, scale=1.0)
                    wds = pa.tile([P, FD // 16], DT.int32, tag="wds")
                    _stt(nc.vector, wds[:], hi[:], 24, lo[:], AL.logical_shift_left, AL.bitwise_or)
                    nc.sync.dma_start(
                        c_r[t0:t0 + CHUNK_T, :, 0:WUSED].rearrange("t p w -> p t w"),
                        wds[:].rearrange("p (t w) -> p t w", t=CHUNK_T))

            tc.strict_bb_all_engine_barrier()

            # ---------------- phase B ----------------
            with tc.tile_pool(name="pb", bufs=2) as pb:
                for ch in (range(E // ECH) if PHASE_B else []):
                    rg = pb.tile([P, EPP, WPR], DT.int32, tag="rg")
                    cg = pb.tile([P, EPP, WPR], DT.int32, tag="cg")
                    i0 = ch * (ECH // 16)
                    i1 = (ch + 1) * (ECH // 16)
                    nc.gpsimd.dma_gather(rg[:], codes[:], idxr_t[:, i0:i1], ECH, ECH, WPR, single_packet=False)
                    nc.gpsimd.dma_gather(cg[:], codes[:], idxc_t[:, i0:i1], ECH, ECH, WPR, single_packet=False)

                    nw = EPP * WUSED  # 1536 real words per partition
                    t = pb.tile([P, nw], DT.int32, tag="t")
                    tv = t[:].rearrange("p (a b) -> p a b", a=EPP)
                    nc.vector.tensor_tensor(out=tv, in0=rg[:, :, 0:WUSED], in1=cg[:, :, 0:WUSED],
                                            op=AL.bitwise_xor)
                    # a = t | t>>1: bit 2j = [field j mismatched] (odd bits garbage,
                    # excluded by the 0x11 masks below)
                    _stt(nc.vector, t[:], t[:], 1, t[:], AL.logical_shift_right, AL.bitwise_or)
                    # nibble counts <=2
                    v2 = pb.tile([P, nw], DT.int32, tag="v2")
                    v1 = pb.tile([P, nw], DT.int32, tag="v1")
                    _ts(nc.vector, v2[:], t[:], 2, AL.logical_shift_right, 0x11111111, AL.bitwise_and)
                    _ts(nc.vector, v1[:], t[:], 0x11111111, AL.bitwise_and, 0, AL.bitwise_or)
                    # int32 DVE adds go through fp32; SWAR sums never carry across
                    # 16-bit halves, so add on int16 views (values < 2^24, exact)
                    nc.vector.tensor_tensor(out=t[:].bitcast(DT.int16), in0=v1[:].bitcast(DT.int16),
                                            in1=v2[:].bitcast(DT.int16), op=AL.add)
                    # byte counts <=4
                    _ts(nc.vector, v2[:], t[:], 4, AL.logical_shift_right, 0x0F0F0F0F, AL.bitwise_and)
                    _ts(nc.vector, v1[:], t[:], 0x0F0F0F0F, AL.bitwise_and, 0, AL.bitwise_or)
                    nc.vector.tensor_tensor(out=t[:].bitcast(DT.int16), in0=v1[:].bitcast(DT.int16),
                                            in1=v2[:].bitcast(DT.int16), op=AL.add)
                    # sum all bytes on ACT (accum_out); counts <= 24576, f32-exact
                    dump = pb.tile([P, EPP * WUSED * 4], DT.int8, tag="dump")
                    nc.scalar.activation(dump[:], t[:].bitcast(DT.int8), AF.Copy,
                                         bias=0.0, scale=1.0, accum_out=partial_t[:, ch:ch + 1])
            nc.sync.dma_start(partials[:], partial_t[:])
    nc.compile()
    return nc


def _affine_scale(means):
    # means[k] = scale*k + bias requires affine means; true for (-mu, 0, mu)?
    # General 3-point: only if means[2]-means[1] == means[1]-means[0].
    return np.float32(means[1] - means[0])


def _affine_bias(means):
    return np.float32(means[0])


def _means_affine(means):
    return np.float32(means[2]) - np.float32(means[1]) == np.float32(means[1]) - np.float32(means[0])


def kernel(x, edge_index, gumbel_noise, state_means, log_stds, transition_logits, start_logits):
    x = np.ascontiguousarray(np.asarray(x, dtype=np.float32))
    g = np.ascontiguousarray(np.asarray(gumbel_noise, dtype=np.float32))
    ei = np.asarray(edge_index)
    sm = np.asarray(state_means, dtype=np.float32)
    ls = np.asarray(log_stds, dtype=np.float32)

    means = np.array([sm[0], np.float32(0.0), sm[1]], dtype=np.float32)
    stds = (np.exp(ls) + np.float32(1e-6)).astype(np.float32)
    stds_equal = bool(np.all(stds == stds[0]))
    r = (np.float32(1.0) / stds).astype(np.float32)
    rp = (r.astype(np.float64) / np.sqrt(np.float64(2.0))).astype(np.float32)
    dk = (-np.log(stds)).astype(np.float32)  # only used when stds differ
    assert _means_affine(means), "non-affine means need a predicated-copy path"

    key = (means.tobytes(), rp.tobytes(), dk.tobytes(), stds_equal)
    if key not in _CACHE:
        _CACHE[key] = _build(means, rp, dk, stds_equal)
    nc = _CACHE[key]

    # host-side input prep
    pat_row = np.where(np.isin(np.arange(FD) % 16, (0, 12)), np.float32(0.0), np.float32(4.0))
    pat_full = np.tile(pat_row[None, :], (P, 1)).astype(np.float32)

    def g_planes(gg, b0, b1):
        pl = np.ascontiguousarray(-gg[:, b0:b1, :].transpose(2, 0, 1))
        if not stds_equal:
            pl = pl - dk.reshape(3, 1, 1).astype(np.float32)
        return np.ascontiguousarray(pl.astype(np.float32))

    def wrap_idx(a):
        # per phase-B chunk of ECH edges: idx j of chunk at [j%16, ch*(ECH//16)+j//16]
        cols = []
        for ch in range(E // ECH):
            blk = a[ch * ECH:(ch + 1) * ECH].reshape(ECH // 16, 16).T
            cols.append(blk)
        return np.tile(np.concatenate(cols, axis=1), (P // 16, 1)).astype(np.int16)

    idx_r = wrap_idx(ei[0].astype(np.int64))
    idx_c = wrap_idx(ei[1].astype(np.int64))

    in_maps = []
    for c in range(NCORES):
        b0, b1 = c * BS, (c + 1) * BS
        in_maps.append({
            "x_s": np.ascontiguousarray(x[:, b0:b1]),
            "g_s": g_planes(g, b0, b1),
            "idx_r": idx_r,
            "idx_c": idx_c,
            "pat": pat_full,
        })

    global LAST_RESULT, LAST_IN_MAPS
    LAST_IN_MAPS = in_maps
    res = bass_utils.run_bass_kernel_spmd(nc, in_maps, core_ids=list(range(NCORES)))
    LAST_RESULT = res
    outs = res.results

    norm_copy = np.concatenate([outs[c]["norm"] for c in range(NCORES)], axis=1)
    total = sum(int(outs[c]["partials"].astype(np.int64).sum()) for c in range(NCORES))
    loss = np.float32(np.float64(0.1) * 2.0 * total / (E * B * 3))
    return norm_copy, loss


# revision 38
# speedup vs baseline: 1.0641x; 1.0443x over previous
"""Trainium2 Bass kernel for nn_DifferentiableHMM_Centered.

Computes, for x (S,B) f32 and gumbel_noise (S,B,3) f32:
  norm_copy[s,b] = all_means[argmax_k y_k]   (straight-through gumbel argmax)
  loss = 0.1 * mean((states[row]-states[col])**2)
       = 0.1 * 2 * #{(e,b): argmax differs} / (E*B*3)
where y_k = (log N(x; mu_k, sigma_k) + g_k)/tau.  The Viterbi trellis in the
reference is dead code (outputs never consume it) and is skipped.

argmax_k y_k == argmax_k (g_k - ((x-mu_k)*r'_k)^2 + d_k), r'_k = 1/(sigma_k*sqrt(2)),
d_k = -log sigma_k (d_k constant when sigmas equal -> dropped).

Sharding: bins split across 8 cores (phase A elementwise is trivially parallel;
phase B edge-mismatch counting is local per bin-slice, edges replicated).
Per core: codes (argmax in {0,1,2}) are packed 2 bits/bin into int32 words
(12+4 bins per word via an fp32 mult-add scan), written to a DRAM table with
256B rows, row-gathered per edge endpoint with dma_gather, XOR'd and
SWAR-popcounted to mismatch counts.
"""
import numpy as np

import concourse.bacc as bacc
import concourse.mybir as mybir
from concourse.tile import TileContext
from concourse import bass_utils
from concourse.library_config import mlp

AL = mybir.AluOpType
AF = mybir.ActivationFunctionType
DT = mybir.dt

S = 4096            # spots
B = 4096            # bins (total)
E = 24576           # edges
NCORES = 8
BS = B // NCORES    # bins per core = 512
P = 128
NG = S // P         # spot groups = 32
CHUNK_T = 2         # spot groups per phase-A chunk
NCHUNK = NG // CHUNK_T
FD = CHUNK_T * BS   # phase-A chunk free dim = 2048
WPR = 64            # int32 words per code row (256B, only first BS//16=32 used)
WUSED = BS // 16    # 32 packed words per row
ECH = E // 8        # edges per phase-B chunk = 3072
EPP = ECH // P      # edge rows per partition per chunk = 48

_CACHE = {}


def _imm(v):
    return mybir.ImmediateValue(dtype=DT.int32, value=int(v))


def _ts(eng, out, in0, s1, op0, s2, op1, accum_out=None):
    """tensor_scalar with int32 immediates; op0/op1 same class (both bitvec)."""
    outs = [eng.lower_ap(out)]
    if accum_out is not None:
        outs.append(eng.lower_ap(accum_out))
    return eng.add_instruction(mybir.InstTensorScalarPtr(
        name=eng.bass.get_next_instruction_name(),
        op0=op0, op1=op1,
        ins=[eng.lower_ap(in0), _imm(s1), _imm(s2)],
        outs=outs))


def _stt(eng, out, in0, s, in1, op0, op1):
    return eng.add_instruction(mybir.InstTensorScalarPtr(
        name=eng.bass.get_next_instruction_name(),
        is_scalar_tensor_tensor=True, op0=op0, op1=op1,
        ins=[eng.lower_ap(in0), _imm(s), eng.lower_ap(in1)],
        outs=[eng.lower_ap(out)]))


PHASE_B = True


def _build(means, rp, dk, stds_equal):
    """Build the SPMD program (same for all cores; bins slice differs only in
    the input data)."""
    nc = bacc.Bacc(None, target_bir_lowering=False, debug=True)

    x_s = nc.dram_tensor("x_s", [S, BS], DT.float32, kind="ExternalInput")
    g_s = nc.dram_tensor("g_s", [3, S, BS], DT.float32, kind="ExternalInput")
    idx_r = nc.dram_tensor("idx_r", [P, E // 16], DT.int16, kind="ExternalInput")
    idx_c = nc.dram_tensor("idx_c", [P, E // 16], DT.int16, kind="ExternalInput")
    pat = nc.dram_tensor("pat", [P, FD], DT.float32, kind="ExternalInput")
    norm = nc.dram_tensor("norm", [S, BS], DT.float32, kind="ExternalOutput")
    partials = nc.dram_tensor("partials", [P, 8], DT.float32, kind="ExternalOutput")
    codes = nc.dram_tensor("codes", [S, WPR], DT.int32)

    x_r = x_s.rearrange("(t p) b -> t p b", p=P)
    g_r = g_s.rearrange("k (t p) b -> k t p b", p=P)
    n_r = norm.rearrange("(t p) b -> t p b", p=P)
    c_r = codes.rearrange("(t p) w -> t p w", p=P)

    with TileContext(nc) as tc:
        nc.gpsimd.load_library(mlp)
        with tc.tile_pool(name="outer", bufs=1) as outer:
            # constants
            bias_t = [outer.tile([P, 1], DT.float32, tag=f"bias{k}", name=f"bias{k}") for k in range(3)]
            for k in range(3):
                nc.vector.memset(bias_t[k][:], float(np.float32(-np.float32(means[k]) * np.float32(rp[k]))))
            pat_t = outer.tile([P, FD], DT.float32, tag="pat")
            nc.sync.dma_start(pat_t[:], pat[:])
            idxr_t = outer.tile([P, E // 16], DT.int16, tag="idxr")
            idxc_t = outer.tile([P, E // 16], DT.int16, tag="idxc")
            nc.sync.dma_start(idxr_t[:], idx_r[:])
            nc.sync.dma_start(idxc_t[:], idx_c[:])
            partial_t = outer.tile([P, 8], DT.float32, tag="partial")
            nc.vector.memset(partial_t[:], 0.0)

            # ---------------- phase A ----------------
            with tc.tile_pool(name="pa", bufs=2) as pa:
                for ci in range(NCHUNK):
                    t0 = ci * CHUNK_T
                    xt = pa.tile([P, CHUNK_T, BS], DT.float32, tag="x", bufs=3)
                    nc.sync.dma_start(xt[:], x_r[t0:t0 + CHUNK_T].rearrange("t p b -> p t b"))

                    # B_k = sq_k + (-g_k): ACT Square, then the gumbel-plane
                    # load DMA accumulates (CCE add) onto it -- the subtraction
                    # costs zero DVE passes.  Host supplies negated planes
                    # (with -d_k folded in when sigmas differ).  argmax b ==
                    # argmin B, so comparisons below invert (min / is_gt).
                    sq = []
                    for k in range(3):
                        s_k = pa.tile([P, CHUNK_T, BS], DT.float32, tag=f"sq{k}", name=f"sq{k}", bufs=3)
                        nc.scalar.activation(s_k[:], xt[:], AF.Square,
                                             bias=bias_t[k][:], scale=float(rp[k]))
                        nc.gpsimd.dma_start(s_k[:], g_r[k, t0:t0 + CHUNK_T].rearrange("t p b -> p t b"),
                                            accum_op=AL.add)
                        sq.append(s_k)

                    # code in {0,1,2}; first-index-wins tie-break:
                    # code = [B0 > min(B1,B2)] * (1 + [B1 > B2])
                    m12 = pa.tile([P, CHUNK_T, BS], DT.float32, tag="m12", bufs=1)
                    nc.vector.tensor_tensor(out=m12[:], in0=sq[1][:], in1=sq[2][:], op=AL.min)
                    c12i = pa.tile([P, CHUNK_T, BS], DT.float32, tag="c12i", bufs=1)
                    nc.vector.tensor_tensor(out=c12i[:], in0=sq[1][:], in1=sq[2][:], op=AL.is_gt)
                    c0i = pa.tile([P, CHUNK_T, BS], DT.float32, tag="c0i", bufs=1)
                    nc.vector.tensor_tensor(out=c0i[:], in0=sq[0][:], in1=m12[:], op=AL.is_gt)
                    code = pa.tile([P, CHUNK_T, BS], DT.float32, tag="code")
                    nc.vector.scalar_tensor_tensor(out=code[:], in0=c12i[:], scalar=1.0,
                                                   in1=c0i[:], op0=AL.add, op1=AL.mult)

                    # norm_copy = means[code] = scale*code + bias (means affine in code
                    # only when means are affine; general: two predicated copies)
                    ncv = pa.tile([P, CHUNK_T, BS], DT.float32, tag="ncv")
                    nc.scalar.activation(ncv[:], code[:], AF.Copy,
                                         bias=float(_affine_bias(means)), scale=float(_affine_scale(means)))
                    nc.sync.dma_start(n_r[t0:t0 + CHUNK_T].rearrange("t p b -> p t b"), ncv[:])

                    # pack codes 2b/bin: scan resets at j%16 in {0,12}
                    sc = pa.tile([P, FD], DT.float32, tag="scan", bufs=1)
                    nc.vector.tensor_tensor_scan(
                        out=sc[:], data0=pat_t[:], data1=code[:].rearrange("p t b -> p (t b)"),
                        initial=0.0, op0=AL.mult, op1=AL.add)
                    lo = pa.tile([P, FD // 16], DT.int32, tag="lo")
                    hi = pa.tile([P, FD // 16], DT.int32, tag="hi")
                    nc.scalar.activation(lo[:], sc[:, 11::16], AF.Copy, bias=0.0, scale=1.0)
                    nc.scalar.activation(hi[:], sc[:, 15::16], AF.Copy, bias=0.0, scale=1.0)
                    wds = pa.tile([P, FD // 16], DT.int32, tag="wds")
                    _stt(nc.vector, wds[:], hi[:], 24, lo[:], AL.logical_shift_left, AL.bitwise_or)
                    nc.sync.dma_start(
                        c_r[t0:t0 + CHUNK_T, :, 0:WUSED].rearrange("t p w -> p t w"),
                        wds[:].rearrange("p (t w) -> p t w", t=CHUNK_T))

            tc.strict_bb_all_engine_barrier()

            # ---------------- phase B ----------------
            with tc.tile_pool(name="pb", bufs=2) as pb:
                for ch in (range(E // ECH) if PHASE_B else []):
                    rg = pb.tile([P, EPP, WPR], DT.int32, tag="rg")
                    cg = pb.tile([P, EPP, WPR], DT.int32, tag="cg")
                    i0 = ch * (ECH // 16)
                    i1 = (ch + 1) * (ECH // 16)
                    nc.gpsimd.dma_gather(rg[:], codes[:], idxr_t[:, i0:i1], ECH, ECH, WPR, single_packet=False)
                    nc.gpsimd.dma_gather(cg[:], codes[:], idxc_t[:, i0:i1], ECH, ECH, WPR, single_packet=False)

                    nw = EPP * WUSED  # 1536 real words per partition
                    t = pb.tile([P, nw], DT.int32, tag="t")
                    tv = t[:].rearrange("p (a b) -> p a b", a=EPP)
                    nc.vector.tensor_tensor(out=tv, in0=rg[:, :, 0:WUSED], in1=cg[:, :, 0:WUSED],
                                            op=AL.bitwise_xor)
                    # a = t | t>>1: bit 2j = [field j mismatched] (odd bits garbage,
                    # excluded by the 0x11 masks below)
                    _stt(nc.vector, t[:], t[:], 1, t[:], AL.logical_shift_right, AL.bitwise_or)
                    # nibble counts <=2
                    v2 = pb.tile([P, nw], DT.int32, tag="v2")
                    v1 = pb.tile([P, nw], DT.int32, tag="v1")
                    _ts(nc.vector, v2[:], t[:], 2, AL.logical_shift_right, 0x11111111, AL.bitwise_and)
                    _ts(nc.vector, v1[:], t[:], 0x11111111, AL.bitwise_and, 0, AL.bitwise_or)
                    # int32 DVE adds go through fp32; SWAR sums never carry across
                    # 16-bit halves, so add on int16 views (values < 2^24, exact)
                    nc.vector.tensor_tensor(out=t[:].bitcast(DT.int16), in0=v1[:].bitcast(DT.int16),
                                            in1=v2[:].bitcast(DT.int16), op=AL.add)
                    # byte counts <=4
                    _ts(nc.vector, v2[:], t[:], 4, AL.logical_shift_right, 0x0F0F0F0F, AL.bitwise_and)
                    _ts(nc.vector, v1[:], t[:], 0x0F0F0F0F, AL.bitwise_and, 0, AL.bitwise_or)
                    nc.vector.tensor_tensor(out=t[:].bitcast(DT.int16), in0=v1[:].bitcast(DT.int16),
                                            in1=v2[:].bitcast(DT.int16), op=AL.add)
                    # sum all bytes on ACT (accum_out); counts <= 24576, f32-exact
                    dump = pb.tile([P, EPP * WUSED * 4], DT.int8, tag="dump")
                    nc.scalar.activation(dump[:], t[:].bitcast(DT.int8), AF.Copy,
                                         bias=0.0, scale=1.0, accum_out=partial_t[:, ch:ch + 1])
            nc.sync.dma_start(partials[:], partial_t[:])
    nc.compile()
    return nc


def _affine_scale(means):
    # means[k] = scale*k + bias requires affine means; true for (-mu, 0, mu)?
    # General 3-point: only if means[2]-means[1] == means[1]-means[0].
    return np.float32(means[1] - means[0])


def _affine_bias(means):
    return np.float32(means[0])


def _means_affine(means):
    return np.float32(means[2]) - np.float32(means[1]) == np.float32(means[1]) - np.float32(means[0])


def kernel(x, edge_index, gumbel_noise, state_means, log_stds, transition_logits, start_logits):
    x = np.ascontiguousarray(np.asarray(x, dtype=np.float32))
    g = np.ascontiguousarray(np.asarray(gumbel_noise, dtype=np.float32))
    ei = np.asarray(edge_index)
    sm = np.asarray(state_means, dtype=np.float32)
    ls = np.asarray(log_stds, dtype=np.float32)

    means = np.array([sm[0], np.float32(0.0), sm[1]], dtype=np.float32)
    stds = (np.exp(ls) + np.float32(1e-6)).astype(np.float32)
    stds_equal = bool(np.all(stds == stds[0]))
    r = (np.float32(1.0) / stds).astype(np.float32)
    rp = (r.astype(np.float64) / np.sqrt(np.float64(2.0))).astype(np.float32)
    dk = (-np.log(stds)).astype(np.float32)  # only used when stds differ
    assert _means_affine(means), "non-affine means need a predicated-copy path"

    key = (means.tobytes(), rp.tobytes(), dk.tobytes(), stds_equal)
    if key not in _CACHE:
        _CACHE[key] = _build(means, rp, dk, stds_equal)
    nc = _CACHE[key]

    # host-side input prep
    pat_row = np.where(np.isin(np.arange(FD) % 16, (0, 12)), np.float32(0.0), np.float32(4.0))
    pat_full = np.tile(pat_row[None, :], (P, 1)).astype(np.float32)

    def g_planes(gg, b0, b1):
        pl = np.ascontiguousarray(-gg[:, b0:b1, :].transpose(2, 0, 1))
        if not stds_equal:
            pl = pl - dk.reshape(3, 1, 1).astype(np.float32)
        return np.ascontiguousarray(pl.astype(np.float32))

    def wrap_idx(a):
        # per phase-B chunk of ECH edges: idx j of chunk at [j%16, ch*(ECH//16)+j//16]
        cols = []
        for ch in range(E // ECH):
            blk = a[ch * ECH:(ch + 1) * ECH].reshape(ECH // 16, 16).T
            cols.append(blk)
        return np.tile(np.concatenate(cols, axis=1), (P // 16, 1)).astype(np.int16)

    idx_r = wrap_idx(ei[0].astype(np.int64))
    idx_c = wrap_idx(ei[1].astype(np.int64))

    in_maps = []
    for c in range(NCORES):
        b0, b1 = c * BS, (c + 1) * BS
        in_maps.append({
            "x_s": np.ascontiguousarray(x[:, b0:b1]),
            "g_s": g_planes(g, b0, b1),
            "idx_r": idx_r,
            "idx_c": idx_c,
            "pat": pat_full,
        })

    global LAST_RESULT, LAST_IN_MAPS
    LAST_IN_MAPS = in_maps
    res = bass_utils.run_bass_kernel_spmd(nc, in_maps, core_ids=list(range(NCORES)))
    LAST_RESULT = res
    outs = res.results

    norm_copy = np.concatenate([outs[c]["norm"] for c in range(NCORES)], axis=1)
    total = sum(int(outs[c]["partials"].astype(np.int64).sum()) for c in range(NCORES))
    loss = np.float32(np.float64(0.1) * 2.0 * total / (E * B * 3))
    return norm_copy, loss
